# revision 1
# baseline (speedup 1.0000x reference)
"""GCN 5-layer message-passing kernel for 8 Trainium2 NeuronCores.

Strategy (node-sharded, dst-partitioned):
- Core c owns dst nodes [c*12500, (c+1)*12500). Edges (incl self-loops) are
  routed to the core owning their dst.
- GCN norm factors dinv[src]*dinv[dst] are folded: dinv[src] pre-scales the
  transformed feature table h2s = dinv * (h @ W) before the gather; dinv[dst]
  scales the per-dst-block epilogue. No per-edge multiplies on device.
- Per layer: each core computes its slice of h2s (transform matmul), cores
  AllGather the full table, then each core gathers rows for its edges via
  GPSIMD dma_gather (random HBM reads) and reduces them per dst block with
  one-hot segment matmuls accumulating in PSUM (scatter-free).
- Edges are sorted by (superblock, src-chunk, dst-block) and padded to
  128-token tiles so int16 gather indices stay in range (4 chunks of 32768
  rows) and every 128-token matmul tile maps to a single dst block.
- Final global mean-pool via one-hot matmul + AllReduce, then the FC layer.
"""
import os
import numpy as np
import ml_dtypes

KSKIP_GATHER = os.environ.get("KSKIP_GATHER", "0") == "1"
KSKIP_MM = os.environ.get("KSKIP_MM", "0") == "1"

KSKIP_GATHER = os.environ.get("KSKIP_GATHER", "0") == "1"
KSKIP_MM = os.environ.get("KSKIP_MM", "0") == "1"

N = 100000
E = 1600000
NCORES = 8
NPC = N // NCORES            # 12500 nodes per core
NB = (NPC + 127) // 128      # 98 dst blocks per core
SB_BLOCKS = 4                # dst blocks per superblock
CHUNK = 32768                # gather src chunk (int16 index range)
NCHUNK = (N + CHUNK - 1) // CHUNK  # 4
NG = 64                      # graphs
F0 = 128
bf16 = ml_dtypes.bfloat16

# layer i: h2s_i width (transformed source), produced by W_i
FW = [128, 128, 128, 64, 64]   # W5 padded 32->64
# h2s dtype per layer: bf16 for 128-wide (256B elems), f32 for 64-wide (256B)
FDT = ["bf16", "bf16", "bf16", "f32", "f32"]

_last_results = None
_last_nc = None
_last_in_maps = None


def _superblocks():
    sbs = []
    b = 0
    while b < NB:
        sbs.append(list(range(b, min(b + SB_BLOCKS, NB))))
        b += SB_BLOCKS
    return sbs


def _prep(x, edge_index, batch, dinv):
    """Build common program structure + per-core data arrays."""
    src = np.concatenate([edge_index[0], np.arange(N, dtype=np.int64)])
    dst = np.concatenate([edge_index[1], np.arange(N, dtype=np.int64)])

    sbs = _superblocks()
    sb_of_block = np.zeros(NB, np.int64)
    for i, sb in enumerate(sbs):
        for b in sb:
            sb_of_block[b] = i

    # per-core sorted edge arrays and per-(block, chunk) counts
    core_edges = []
    counts = np.zeros((NCORES, NB, NCHUNK), np.int64)
    for c in range(NCORES):
        lo = c * NPC
        sel = (dst >= lo) & (dst < lo + NPC)
        es = src[sel]
        ed = dst[sel] - lo
        blk = ed >> 7
        slot = ed & 127
        chk = es >> 15
        order = np.lexsort((es, blk, chk, sb_of_block[blk]))
        es, blk, slot, chk = es[order], blk[order], slot[order], chk[order]
        np.add.at(counts[c], (blk, chk), 1)
        core_edges.append((es, blk, slot, chk))

    # common padded tile counts per (block, chunk): max over cores
    ptiles = (counts.max(axis=0) + 127) // 128     # [NB, NCHUNK] in tiles

    # traversal order: (sb, chunk, block) -> token offsets
    goff = np.zeros((NB, NCHUNK), np.int64)
    sb_meta = []   # per sb: (tok_off, ntok, [(k, off_k, n_k)], [(b, lb, [(tile_off, ntiles)])])
    tok = 0
    for sb in sbs:
        sb_off = tok
        chunk_runs = []
        block_tiles = {b: [] for b in sb}
        for k in range(NCHUNK):
            k_off = tok
            for b in sb:
                goff[b, k] = tok
                nt = int(ptiles[b, k])
                if nt:
                    block_tiles[b].append(((tok - sb_off) // 128, nt))
                tok += nt * 128
            n_k = tok - k_off
            if n_k:
                chunk_runs.append((k, k_off - sb_off, n_k))
        sb_meta.append((sb_off, tok - sb_off, chunk_runs,
                        [(b, lb, block_tiles[b]) for lb, b in enumerate(sb)]))
    T = tok

    # per-core data arrays in the common layout
    core_data = []
    for c in range(NCORES):
        es, blk, slot, chk = core_edges[c]
        gkey = blk * NCHUNK + chk
        if len(es):
            starts = np.r_[0, np.flatnonzero(np.diff(gkey)) + 1]
            runlen = np.diff(np.r_[starts, len(es)])
            rank = np.arange(len(es)) - np.repeat(starts, runlen)
        else:
            rank = np.zeros(0, np.int64)
        pos = goff[blk, chk] + rank
        idx16 = np.zeros(T, np.int16)
        slotv = np.full(T, 999.0, np.float32)
        idx16[pos] = (es - chk * CHUNK).astype(np.int16)
        slotv[pos] = slot
        gidx_w = np.tile(idx16.reshape(T // 16, 16).T, (8, 1))
        dstloc_w = slotv.reshape(T // 128, 128).T.copy()
        core_data.append((gidx_w, dstloc_w))

    # per-core node-level arrays
    node_data = []
    for c in range(NCORES):
        lo = c * NPC
        xT = np.ascontiguousarray(x[lo:lo + NPC].T)              # [128, NPC]
        dre = np.broadcast_to(dinv[lo:lo + NPC], (128, NPC)).copy()
        dpad = np.ones(NB * 128, np.float32)
        dpad[:NPC] = dinv[lo:lo + NPC]
        dcol = dpad.reshape(NB, 128).T.copy()                    # [128, NB]
        bpad = np.full(NB * 128, 999.0, np.float32)
        bpad[:NPC] = batch[lo:lo + NPC].astype(np.float32)
        bloc = bpad.reshape(NB, 128).T.copy()                    # [128, NB]
        node_data.append((xT, dre, dcol, bloc))

    return sb_meta, T, core_data, node_data


def _build_program(sb_meta, T, repeats=1):
    import concourse.bass as bass
    import concourse.bacc as bacc
    import concourse.tile as tile
    from concourse import mybir
    dt = mybir.dt
    Alu = mybir.AluOpType
    Act = mybir.ActivationFunctionType

    nc = bacc.Bacc("TRN2", target_bir_lowering=False, debug=False,
                   num_devices=NCORES)

    # ---- IO ----
    xT_d = nc.dram_tensor("xT", [128, NPC], dt.float32, kind="ExternalInput")
    gidx_d = nc.dram_tensor("gidx", [128, T // 16], dt.int16, kind="ExternalInput")
    dloc_d = nc.dram_tensor("dloc", [128, T // 128], dt.float32, kind="ExternalInput")
    dre_d = nc.dram_tensor("dinvrep", [128, NPC], dt.float32, kind="ExternalInput")
    dcol_d = nc.dram_tensor("dinvcol", [128, NB], dt.float32, kind="ExternalInput")
    bloc_d = nc.dram_tensor("batchloc", [128, NB], dt.float32, kind="ExternalInput")
    iota_d = nc.dram_tensor("iota", [128, 128], dt.float32, kind="ExternalInput")
    iog_d = nc.dram_tensor("iota64", [128, NG], dt.float32, kind="ExternalInput")
    W_d = [nc.dram_tensor(f"W{i+1}", [128 if i == 0 else FW[i - 1], FW[i]],
                          dt.float32, kind="ExternalInput") for i in range(5)]
    bias_d = [nc.dram_tensor(f"b{i+1}", [128, 1], dt.float32, kind="ExternalInput")
              for i in range(4)]
    b5r_d = nc.dram_tensor("b5rep", [128, 64], dt.float32, kind="ExternalInput")
    wfc_d = nc.dram_tensor("Wfc", [32, 10], dt.float32, kind="ExternalInput")
    invc_d = nc.dram_tensor("invcrep", [32, NG], dt.float32, kind="ExternalInput")
    bfc_d = nc.dram_tensor("bfcrep", [NG, 10], dt.float32, kind="ExternalInput")
    out_d = nc.dram_tensor("out", [NG, 10], dt.float32, kind="ExternalOutput")

    RG = [list(range(NCORES))]

    with tile.TileContext(nc) as tc:
        with tc.tile_pool(name="cst", bufs=1) as cst, \
             tc.tile_pool(name="sb", bufs=2) as sbp, \
             tc.tile_pool(name="ps", bufs=4, space="PSUM") as ps, \
             tc.tile_pool(name="ps2", bufs=2, space="PSUM") as ps2, \
             tc.tile_pool(name="psp", bufs=1, space="PSUM") as psp, \
             tc.tile_pool(name="dram", bufs=1, space="DRAM") as dram:

            # ---- constants to SBUF ----
            def cload(name, dten, shape, dtype):
                t = cst.tile(shape, dtype, tag=name)
                nc.sync.dma_start(out=t[:], in_=dten[:])
                return t
            iota_t = cload("iota", iota_d, [128, 128], dt.float32)
            iog_t = cload("iog", iog_d, [128, NG], dt.float32)
            W_t = [cload(f"W{i}", W_d[i], list(W_d[i].shape), dt.float32)
                   for i in range(5)]
            bias_t = [cload(f"b{i}", bias_d[i], [128, 1], dt.float32)
                      for i in range(4)]
            b5r_t = cload("b5r", b5r_d, [128, 64], dt.float32)
            wfc_t = cload("wfc", wfc_d, [32, 10], dt.float32)
            invc_t = cload("invc", invc_d, [32, NG], dt.float32)
            bfc_t = cload("bfc", bfc_d, [NG, 10], dt.float32)
            dcol_t = cload("dcol", dcol_d, [128, NB], dt.float32)
            bloc_t = cload("bloc", bloc_d, [128, NB], dt.float32)

            # ---- DRAM internals ----
            def h2s_tiles(i):
                w, d = FW[i], (dt.bfloat16 if FDT[i] == "bf16" else dt.float32)
                own = dram.tile([NPC, w], d, tag=f"own{i}")
                full = dram.tile([N, w], d, tag=f"full{i}")
                return own, full
            h2s = [h2s_tiles(i) for i in range(5)]
            ar_i = dram.tile([32, NG], dt.float32, tag="ari")
            ar_o = dram.tile([32, NG], dt.float32, tag="aro")

            sbs = _superblocks()
            psum_pool_t = psp.tile([32, NG], dt.float32, tag="pool")
            for _rep in range(repeats):

                # ---- layer 1 transform: h2s_0 = dinv * (x @ W1), bf16 ----
                for si, sb in enumerate(sbs):
                    c0 = sb[0] * 128
                    cn = min(NPC, (sb[-1] + 1) * 128) - c0
                    xsl = sbp.tile([128, SB_BLOCKS * 128], dt.float32, tag="xsl")
                    nc.sync.dma_start(out=xsl[:, :cn], in_=xT_d[:, c0:c0 + cn])
                    for lb, b in enumerate(sb):
                        rows = min(128, NPC - b * 128)
                        pt = ps2.tile([128, FW[0]], dt.float32, tag="tf")
                        nc.tensor.matmul(pt[:], xsl[:, lb * 128:lb * 128 + 128],
                                         W_t[0][:], start=True, stop=True)
                        st = sbp.tile([128, FW[0]], dt.bfloat16, tag="h2st")
                        nc.vector.tensor_scalar(
                            out=st[:], in0=pt[:], scalar1=dcol_t[:, b:b + 1],
                            scalar2=None, op0=Alu.mult)
                        nc.sync.dma_start(out=h2s[0][0][b * 128:b * 128 + rows, :],
                                          in_=st[:rows, :])
                nc.gpsimd.collective_compute(
                    "AllGather", Alu.bypass, replica_groups=RG,
                    ins=[h2s[0][0].opt()], outs=[h2s[0][1].opt()])

                # ---- layers 1..5: gather + segment-matmul + epilogue (+transform) ----
                for i in range(5):
                    w = FW[i]
                    is_bf = FDT[i] == "bf16"
                    mdt = dt.bfloat16 if is_bf else dt.float32
                    src_full = h2s[i][1]
                    for si, sb in enumerate(sbs):
                        sb_off, ntok, chunk_runs, blocks = sb_meta[si]
                        nt_sb = ntok // 128
                        # metadata slabs
                        gix = sbp.tile([128, ntok // 16], dt.int16, tag="gix")
                        nc.sync.dma_start(
                            out=gix[:], in_=gidx_d[:, sb_off // 16:(sb_off + ntok) // 16])
                        dsl = sbp.tile([128, nt_sb], dt.float32, tag="dsl")
                        nc.sync.dma_start(
                            out=dsl[:], in_=dloc_d[:, sb_off // 128:(sb_off + ntok) // 128])
                        # gathers per chunk
                        msg = sbp.tile([128, nt_sb, w], mdt, tag="msg")
                        if KSKIP_GATHER:
                            nc.gpsimd.memset(msg[:], 0.25)
                        GMAX = 1024  # SWDGE ring cap per dma_gather
                        for (k, off_k, n_k) in chunk_runs:
                            rows_k = min(CHUNK, N - k * CHUNK)
                            for p0 in (range(0, n_k, GMAX) if not KSKIP_GATHER else []):
                                pn = min(GMAX, n_k - p0)
                                o = off_k + p0
                                nc.gpsimd.dma_gather(
                                    out_ap=msg[:, o // 128:(o + pn) // 128, :],
                                    in_ap=src_full[k * CHUNK:k * CHUNK + rows_k, :],
                                    idxs_ap=gix[:, o // 16:(o + pn) // 16],
                                    num_idxs=pn, num_idxs_reg=pn, elem_size=w,
                                    single_packet=False)
                        if KSKIP_MM:
                            continue
                        # S build (one wide DVE op)
                        S = sbp.tile([128, nt_sb, 128], dt.bfloat16, tag="S")
                        nc.vector.scalar_tensor_tensor(
                            out=S[:], in0=iota_t[:].unsqueeze(1).broadcast_to([128, nt_sb, 128]),
                            scalar=0.0,
                            in1=dsl[:].unsqueeze(2).broadcast_to([128, nt_sb, 128]),
                            op0=Alu.bypass, op1=Alu.is_equal)
                        if not is_bf:
                            m16 = sbp.tile([128, nt_sb, w], dt.bfloat16, tag="m16")
                            nc.vector.tensor_copy(out=m16[:], in_=msg[:])
                            mm = m16
                        else:
                            mm = msg
                        # dinvrep slab for the superblock (epilogue scaling)
                        c0 = sb[0] * 128
                        cn = min(NPC, (sb[-1] + 1) * 128) - c0
                        drs = sbp.tile([128, SB_BLOCKS * 128], dt.float32, tag="drs")
                        nc.sync.dma_start(out=drs[:, :cn], in_=dre_d[:, c0:c0 + cn])
                        for (b, lb, tiles) in blocks:
                            rows = min(128, NPC - b * 128)
                            tlast = tiles[-1][0] + tiles[-1][1] - 1
                            if i < 4:
                                # transposed aggregation: psum [w, 128dst]
                                pa = ps.tile([w, 128], dt.float32, tag="agg")
                                first = True
                                for (toff, ntl) in tiles:
                                    for t in range(toff, toff + ntl):
                                        nc.tensor.matmul(
                                            pa[:], mm[:, t, :], S[:, t, :],
                                            start=first, stop=(t == tlast))
                                        first = False
                                # epilogue: hT = relu(pa * dinvrep + bias)
                                tmp = sbp.tile([w, 128], dt.float32, tag="tmp")
                                if rows < 128:
                                    nc.vector.memset(tmp[:, rows:], 0.0)
                                nc.vector.tensor_mul(
                                    tmp[:, :rows], pa[:, :rows],
                                    drs[:w, lb * 128:lb * 128 + rows])
                                hT = sbp.tile([w, 128], dt.float32, tag="hT")
                                nc.scalar.activation(hT[:], tmp[:], Act.Relu,
                                                     bias=bias_t[i][:w, 0:1], scale=1.0)
                                # transform: psum2 [128n, w2] = hT.T @ W_{i+1}
                                w2 = FW[i + 1]
                                pt = ps2.tile([128, w2], dt.float32, tag="tf")
                                nc.tensor.matmul(pt[:], hT[:], W_t[i + 1][:],
                                                 start=True, stop=True)
                                odt = dt.bfloat16 if FDT[i + 1] == "bf16" else dt.float32
                                st = sbp.tile([128, w2], odt, tag="h2st")
                                nc.vector.tensor_scalar(
                                    out=st[:], in0=pt[:], scalar1=dcol_t[:, b:b + 1],
                                    scalar2=None, op0=Alu.mult)
                                nc.sync.dma_start(
                                    out=h2s[i + 1][0][b * 128:b * 128 + rows, :],
                                    in_=st[:rows, :])
                            else:
                                # normal aggregation: psum [128dst, 64]
                                pa = ps.tile([128, w], dt.float32, tag="agg")
                                first = True
                                for (toff, ntl) in tiles:
                                    for t in range(toff, toff + ntl):
                                        nc.tensor.matmul(
                                            pa[:], S[:, t, :], mm[:, t, :],
                                            start=first, stop=(t == tlast))
                                        first = False
                                # epilogue: h5 = relu(pa * dinvcol + b5rep)
                                tmp = sbp.tile([128, w], dt.float32, tag="tmp5")
                                nc.vector.scalar_tensor_tensor(
                                    out=tmp[:], in0=pa[:], scalar=dcol_t[:, b:b + 1],
                                    in1=b5r_t[:], op0=Alu.mult, op1=Alu.add)
                                h5 = sbp.tile([128, w], dt.float32, tag="h5")
                                nc.scalar.activation(h5[:], tmp[:], Act.Relu)
                                # pooling: psum_pool [32, NG] += h5[:, :32].T @ B
                                h516 = sbp.tile([128, w], dt.bfloat16, tag="h516")
                                nc.vector.tensor_copy(out=h516[:], in_=h5[:])
                                B = sbp.tile([128, NG], dt.bfloat16, tag="B")
                                nc.vector.tensor_scalar(
                                    out=B[:], in0=iog_t[:], scalar1=bloc_t[:, b:b + 1],
                                    scalar2=None, op0=Alu.is_equal)
                                nc.tensor.matmul(
                                    psum_pool_t[:], h516[:, 0:32], B[:],
                                    start=(si == 0 and lb == 0),
                                    stop=(si == len(sbs) - 1 and lb == len(sb) - 1))
                    if i < 4:
                        nc.gpsimd.collective_compute(
                            "AllGather", Alu.bypass, replica_groups=RG,
                            ins=[h2s[i + 1][0].opt()], outs=[h2s[i + 1][1].opt()])

                # ---- pooling tail: AllReduce, scale, FC ----
                pl = sbp.tile([32, NG], dt.float32, tag="pl")
                nc.vector.tensor_copy(out=pl[:], in_=psum_pool_t[:])
                nc.sync.dma_start(out=ar_i[:], in_=pl[:])
                nc.gpsimd.collective_compute(
                    "AllReduce", Alu.add, replica_groups=RG,
                    ins=[ar_i.opt()], outs=[ar_o.opt()])
                pls = sbp.tile([32, NG], dt.float32, tag="pls")
                nc.sync.dma_start(out=pls[:], in_=ar_o[:])
                plsc = sbp.tile([32, NG], dt.float32, tag="plsc")
                nc.vector.tensor_mul(plsc[:], pls[:], invc_t[:])
                pf = psp.tile([NG, 10], dt.float32, tag="fc")
                nc.tensor.matmul(pf[:], plsc[:], wfc_t[:], start=True, stop=True)
                ot = sbp.tile([NG, 10], dt.float32, tag="ot")
                nc.vector.tensor_add(ot[:], pf[:], bfc_t[:])
                nc.sync.dma_start(out=out_d[:], in_=ot[:])

    nc.compile()
    return nc


def kernel(x, edge_index, batch, W1, b1, W2, b2, W3, b3, W4, b4, W5, b5,
           Wfc, bfc):
    global _last_results, _last_nc, _last_in_maps
    from concourse.bass_utils import run_bass_kernel_spmd

    x = np.asarray(x, np.float32)
    edge_index = np.asarray(edge_index, np.int64)
    batch = np.asarray(batch, np.int64)

    src = np.concatenate([edge_index[0], np.arange(N, dtype=np.int64)])
    dst = np.concatenate([edge_index[1], np.arange(N, dtype=np.int64)])
    deg = np.bincount(dst, minlength=N).astype(np.float32)
    dinv = np.where(deg > 0, 1.0 / np.sqrt(deg), 0.0).astype(np.float32)

    sb_meta, T, core_data, node_data = _prep(x, edge_index, batch, dinv)

    # weights: W5/b5 padded to 64 outputs
    W5p = np.zeros((64, 64), np.float32)
    W5p[:, :32] = np.asarray(W5, np.float32)
    b5p = np.zeros(64, np.float32)
    b5p[:32] = np.asarray(b5, np.float32)
    Ws = [np.asarray(W1, np.float32), np.asarray(W2, np.float32),
          np.asarray(W3, np.float32), np.asarray(W4, np.float32), W5p]
    bs = []
    for b_ in (b1, b2, b3, b4):
        bp = np.zeros((128, 1), np.float32)
        v = np.asarray(b_, np.float32).ravel()
        bp[:v.shape[0], 0] = v
        bs.append(bp)
    b5rep = np.broadcast_to(b5p, (128, 64)).copy()
    cnt = np.bincount(batch, minlength=NG).astype(np.float32)
    invc = (1.0 / np.maximum(cnt, 1.0)).astype(np.float32)
    invc_rep = np.broadcast_to(invc, (32, NG)).copy()
    bfc_rep = np.broadcast_to(np.asarray(bfc, np.float32), (NG, 10)).copy()
    iota = np.broadcast_to(np.arange(128, dtype=np.float32), (128, 128)).copy()
    iota64 = np.broadcast_to(np.arange(NG, dtype=np.float32), (128, NG)).copy()

    nc = _build_program(sb_meta, T, repeats=int(os.environ.get('KREPEATS', '1')))

    in_maps = []
    for c in range(NCORES):
        gidx_w, dstloc_w = core_data[c]
        xT, dre, dcol, bloc = node_data[c]
        im = {"xT": xT, "gidx": gidx_w, "dloc": dstloc_w, "dinvrep": dre,
              "dinvcol": dcol, "batchloc": bloc, "iota": iota, "iota64": iota64,
              "b5rep": b5rep, "Wfc": np.asarray(Wfc, np.float32),
              "invcrep": invc_rep, "bfcrep": bfc_rep}
        for i in range(5):
            im[f"W{i+1}"] = Ws[i]
        for i in range(4):
            im[f"b{i+1}"] = bs[i]
        in_maps.append(im)

    _last_nc = nc
    _last_in_maps = in_maps
    res = run_bass_kernel_spmd(nc, in_maps, core_ids=list(range(NCORES)))
    _last_results = res
    return np.asarray(res.results[0]["out"], np.float32)



# revision 6
# speedup vs baseline: 1.2207x; 1.2207x over previous
"""GCN 5-layer message-passing kernel for 8 Trainium2 NeuronCores.

Strategy (node-sharded, dst-partitioned), v1:
- Core c owns dst nodes [c*12500, (c+1)*12500). Non-self-loop edges are
  routed to the core owning their dst. Self-loop contributions are folded
  into a per-block identity matmul on locally-resident own-slice data
  (no gather, no tokens for the 12500 self loops per core).
- GCN norm factors dinv[src]*dinv[dst] are folded: dinv[src] pre-scales the
  transformed feature table h2s = dinv * (h @ W) before the gather; dinv[dst]
  scales the per-dst-block epilogue.
- Per layer: each core computes its slice of h2s (transform matmul), cores
  AllGather the full table, then each core gathers rows for its edges via
  GPSIMD dma_gather (random HBM reads) and reduces them per dst block with
  one-hot segment matmuls accumulating in PSUM (scatter-free).
- Edges are sorted by (superblock, src-chunk, dst-block, src) and padded to
  a 128 multiple only per (superblock, chunk) segment (cross-core max), so
  padding overhead is ~8% instead of ~30%. Tiles may straddle dst blocks;
  the one-hot S for block lb is built only over lb's tile range with a
  shifted iota (iota + 128*lb == slot512), which self-masks other blocks.
- Final global mean-pool via one-hot matmul + AllReduce, then the FC layer.
"""
import os
import numpy as np
import ml_dtypes

KSKIP_GATHER = os.environ.get("KSKIP_GATHER", "0") == "1"
KSKIP_MM = os.environ.get("KSKIP_MM", "0") == "1"
GMAX = int(os.environ.get("KGMAX", "1024"))  # SWDGE ring cap per dma_gather

N = 100000
E = 1600000
NCORES = 8
NPC = N // NCORES            # 12500 nodes per core
NB = (NPC + 127) // 128      # 98 dst blocks per core
SB_BLOCKS = 4                # dst blocks per superblock
CHUNK = 32768                # gather src chunk (int16 index range)
NCHUNK = (N + CHUNK - 1) // CHUNK  # 4
NG = 64                      # graphs
F0 = 128
bf16 = ml_dtypes.bfloat16

# layer i: h2s_i width (transformed source), produced by W_i
FW = [128, 128, 128, 64, 64]   # W5 padded 32->64
# h2s dtype per layer: bf16 for 128-wide (256B elems), f32 for 64-wide (256B)
FDT = ["bf16", "bf16", "bf16", "f32", "f32"]

_last_results = None
_last_nc = None
_last_in_maps = None


def _superblocks():
    sbs = []
    b = 0
    while b < NB:
        sbs.append(list(range(b, min(b + SB_BLOCKS, NB))))
        b += SB_BLOCKS
    return sbs


def _prep(x, edge_index, batch, dinv):
    """Build common program structure + per-core data arrays.

    Token layout: (sb, chunk) segments, each padded to a 128 multiple of the
    cross-core max token count. Within a segment tokens are sorted by
    (dst block, src). Self-loop edges are NOT tokenized.
    """
    src = edge_index[0]
    dst = edge_index[1]

    sbs = _superblocks()
    nsb = len(sbs)

    # --- per-core sorted edges + per-(sb, chunk) counts ---
    core_edges = []
    counts = np.zeros((NCORES, nsb, NCHUNK), np.int64)
    for c in range(NCORES):
        lo = c * NPC
        sel = (dst >= lo) & (dst < lo + NPC)
        es = src[sel]
        ed = dst[sel] - lo
        blk = ed >> 7
        sb = blk // SB_BLOCKS
        slot = ed - sb * (SB_BLOCKS * 128)   # slot within superblock [0,512)
        chk = es >> 15
        order = np.lexsort((es, blk, chk, sb))
        es, sb, blk, slot, chk = (es[order], sb[order], blk[order],
                                  slot[order], chk[order])
        np.add.at(counts[c], (sb, chk), 1)
        core_edges.append((es, sb, blk, slot, chk))

    # segment lengths: cross-core max, padded to 128
    Lseg = ((counts.max(axis=0) + 127) // 128) * 128   # [nsb, NCHUNK]

    # --- traversal order: (sb, chunk) -> token offsets ---
    seg_off = np.zeros((nsb, NCHUNK), np.int64)
    tok = 0
    sb_tok_off = np.zeros(nsb, np.int64)
    for si in range(nsb):
        sb_tok_off[si] = tok
        for k in range(NCHUNK):
            seg_off[si, k] = tok
            tok += int(Lseg[si, k])
    T = tok

    # --- per-core data arrays in the common layout ---
    core_data = []
    boundaries = np.zeros((NCORES, nsb, NCHUNK, SB_BLOCKS + 1), np.int64)
    for c in range(NCORES):
        es, sb, blk, slot, chk = core_edges[c]
        key = sb * NCHUNK + chk
        if len(es):
            starts = np.r_[0, np.flatnonzero(np.diff(key)) + 1]
            runlen = np.diff(np.r_[starts, len(es)])
            rank = np.arange(len(es)) - np.repeat(starts, runlen)
        else:
            rank = np.zeros(0, np.int64)
        pos = seg_off[sb, chk] + rank
        idx16 = np.zeros(T, np.int16)
        slotv = np.full(T, 999.0, np.float32)
        idx16[pos] = (es - chk * CHUNK).astype(np.int16)
        slotv[pos] = slot
        lb = blk - sb * SB_BLOCKS
        np.add.at(boundaries[c], (sb, chk, lb + 1), 1)
        gidx_w = np.tile(idx16.reshape(T // 16, 16).T, (8, 1))
        dstloc_w = slotv.reshape(T // 128, 128).T.copy()
        core_data.append((gidx_w, dstloc_w))
    boundaries = np.cumsum(boundaries, axis=3)  # [c, sb, chk, lb+1] prefix

    # --- per (sb, chunk, lb): union tile range + matmul entries ---
    # sranges[si]: dict (k, lb) -> (t0, t1) sb-local tile range (inclusive)
    # sb_mm[si]: ordered list of (t_local, lb, k)
    sranges = []
    sb_mm = []
    for si in range(nsb):
        nblk = len(sbs[si])
        rng = {}
        entries = []
        for k in range(NCHUNK):
            base = int(seg_off[si, k] - sb_tok_off[si])
            for lb in range(nblk):
                s = int(boundaries[:, si, k, lb].min(axis=0))
                e = int(boundaries[:, si, k, lb + 1].max(axis=0))
                if e <= s:
                    continue
                t0 = (base + s) // 128
                t1 = (base + e - 1) // 128
                rng[(k, lb)] = (t0, t1)
                for t in range(t0, t1 + 1):
                    entries.append((t, lb, k))
        entries.sort()
        sranges.append(rng)
        sb_mm.append(entries)

    # --- per-core node-level arrays ---
    node_data = []
    for c in range(NCORES):
        lo = c * NPC
        xT = np.ascontiguousarray(x[lo:lo + NPC].T)              # [128, NPC]
        dre = np.broadcast_to(dinv[lo:lo + NPC], (128, NPC)).copy()
        dpad = np.ones(NB * 128, np.float32)
        dpad[:NPC] = dinv[lo:lo + NPC]
        dcol = dpad.reshape(NB, 128).T.copy()                    # [128, NB]
        bpad = np.full(NB * 128, 999.0, np.float32)
        bpad[:NPC] = batch[lo:lo + NPC].astype(np.float32)
        bloc = bpad.reshape(NB, 128).T.copy()                    # [128, NB]
        node_data.append((xT, dre, dcol, bloc))

    meta = dict(sbs=sbs, Lseg=Lseg, seg_off=seg_off, sb_tok_off=sb_tok_off,
                sranges=sranges, sb_mm=sb_mm, T=T)
    return meta, core_data, node_data


def _build_program(meta, repeats=1):
    import concourse.bass as bass
    import concourse.bacc as bacc
    import concourse.tile as tile
    from concourse import mybir
    dt = mybir.dt
    Alu = mybir.AluOpType
    Act = mybir.ActivationFunctionType

    T = meta["T"]
    sbs = meta["sbs"]
    seg_off = meta["seg_off"]
    sb_tok_off = meta["sb_tok_off"]
    Lseg = meta["Lseg"]
    sranges = meta["sranges"]
    sb_mm = meta["sb_mm"]
    nsb = len(sbs)

    nc = bacc.Bacc("TRN2", target_bir_lowering=False, debug=False,
                   num_devices=NCORES)

    # ---- IO ----
    xT_d = nc.dram_tensor("xT", [128, NPC], dt.float32, kind="ExternalInput")
    gidx_d = nc.dram_tensor("gidx", [128, T // 16], dt.int16, kind="ExternalInput")
    dloc_d = nc.dram_tensor("dloc", [128, T // 128], dt.float32, kind="ExternalInput")
    dre_d = nc.dram_tensor("dinvrep", [128, NPC], dt.float32, kind="ExternalInput")
    dcol_d = nc.dram_tensor("dinvcol", [128, NB], dt.float32, kind="ExternalInput")
    bloc_d = nc.dram_tensor("batchloc", [128, NB], dt.float32, kind="ExternalInput")
    iota_d = nc.dram_tensor("iota", [128, 128], dt.float32, kind="ExternalInput")
    id16_d = nc.dram_tensor("ident16", [128, 128], dt.bfloat16, kind="ExternalInput")
    iog_d = nc.dram_tensor("iota64", [128, NG], dt.float32, kind="ExternalInput")
    W_d = [nc.dram_tensor(f"W{i+1}", [128 if i == 0 else FW[i - 1], FW[i]],
                          dt.float32, kind="ExternalInput") for i in range(5)]
    bias_d = [nc.dram_tensor(f"b{i+1}", [128, 1], dt.float32, kind="ExternalInput")
              for i in range(4)]
    b5r_d = nc.dram_tensor("b5rep", [128, 64], dt.float32, kind="ExternalInput")
    wfc_d = nc.dram_tensor("Wfc", [32, 10], dt.float32, kind="ExternalInput")
    invc_d = nc.dram_tensor("invcrep", [32, NG], dt.float32, kind="ExternalInput")
    bfc_d = nc.dram_tensor("bfcrep", [NG, 10], dt.float32, kind="ExternalInput")
    out_d = nc.dram_tensor("out", [NG, 10], dt.float32, kind="ExternalOutput")

    RG = [list(range(NCORES))]

    with tile.TileContext(nc) as tc:
        with tc.tile_pool(name="cst", bufs=1) as cst, \
             tc.tile_pool(name="sb", bufs=2) as sbp, \
             tc.tile_pool(name="ps", bufs=1, space="PSUM") as ps, \
             tc.tile_pool(name="ps2", bufs=2, space="PSUM") as ps2, \
             tc.tile_pool(name="psp", bufs=1, space="PSUM") as psp, \
             tc.tile_pool(name="dram", bufs=1, space="DRAM") as dram:

            # ---- constants to SBUF ----
            def cload(name, dten, shape, dtype):
                t = cst.tile(shape, dtype, tag=name)
                nc.sync.dma_start(out=t[:], in_=dten[:])
                return t
            iota_t = cload("iota", iota_d, [128, 128], dt.float32)
            id16_t = cload("id16", id16_d, [128, 128], dt.bfloat16)
            iog_t = cload("iog", iog_d, [128, NG], dt.float32)
            W_t = [cload(f"W{i}", W_d[i], list(W_d[i].shape), dt.float32)
                   for i in range(5)]
            bias_t = [cload(f"b{i}", bias_d[i], [128, 1], dt.float32)
                      for i in range(4)]
            b5r_t = cload("b5r", b5r_d, [128, 64], dt.float32)
            wfc_t = cload("wfc", wfc_d, [32, 10], dt.float32)
            invc_t = cload("invc", invc_d, [32, NG], dt.float32)
            bfc_t = cload("bfc", bfc_d, [NG, 10], dt.float32)
            dcol_t = cload("dcol", dcol_d, [128, NB], dt.float32)
            bloc_t = cload("bloc", bloc_d, [128, NB], dt.float32)

            # ---- DRAM internals ----
            def h2s_tiles(i):
                w, d = FW[i], (dt.bfloat16 if FDT[i] == "bf16" else dt.float32)
                own = dram.tile([NPC, w], d, tag=f"own{i}")
                full = dram.tile([N, w], d, tag=f"full{i}")
                return own, full
            h2s = [h2s_tiles(i) for i in range(5)]
            ar_i = dram.tile([32, NG], dt.float32, tag="ari")
            ar_o = dram.tile([32, NG], dt.float32, tag="aro")

            psum_pool_t = psp.tile([32, NG], dt.float32, tag="pool")
            for _rep in range(repeats):

                # ---- layer 1 transform: h2s_0 = dinv * (x @ W1), bf16 ----
                for si, sb in enumerate(sbs):
                    c0 = sb[0] * 128
                    cn = min(NPC, (sb[-1] + 1) * 128) - c0
                    xsl = sbp.tile([128, SB_BLOCKS * 128], dt.float32, tag="xsl")
                    nc.sync.dma_start(out=xsl[:, :cn], in_=xT_d[:, c0:c0 + cn])
                    for lb, b in enumerate(sb):
                        rows = min(128, NPC - b * 128)
                        pt = ps2.tile([128, FW[0]], dt.float32, tag="tf")
                        nc.tensor.matmul(pt[:], xsl[:, lb * 128:lb * 128 + 128],
                                         W_t[0][:], start=True, stop=True)
                        st = sbp.tile([128, FW[0]], dt.bfloat16, tag="h2st")
                        nc.vector.tensor_scalar(
                            out=st[:], in0=pt[:], scalar1=dcol_t[:, b:b + 1],
                            scalar2=None, op0=Alu.mult)
                        nc.sync.dma_start(out=h2s[0][0][b * 128:b * 128 + rows, :],
                                          in_=st[:rows, :])
                nc.gpsimd.collective_compute(
                    "AllGather", Alu.bypass, replica_groups=RG,
                    ins=[h2s[0][0].opt()], outs=[h2s[0][1].opt()])

                # ---- layers 1..5: gather + segment-matmul + epilogue ----
                for i in range(5):
                    w = FW[i]
                    is_bf = FDT[i] == "bf16"
                    mdt = dt.bfloat16 if is_bf else dt.float32
                    src_full = h2s[i][1]
                    for si, sb in enumerate(sbs):
                        sb_off = int(sb_tok_off[si])
                        ntok = int(Lseg[si].sum())
                        nt_sb = ntok // 128
                        # metadata slabs
                        gix = sbp.tile([128, ntok // 16], dt.int16, tag="gix")
                        nc.sync.dma_start(
                            out=gix[:], in_=gidx_d[:, sb_off // 16:(sb_off + ntok) // 16])
                        dsl = sbp.tile([128, nt_sb], dt.float32, tag="dsl")
                        nc.sync.dma_start(
                            out=dsl[:], in_=dloc_d[:, sb_off // 128:(sb_off + ntok) // 128])
                        # gathers per (chunk) segment, GMAX-batched
                        msg = sbp.tile([128, nt_sb, w], mdt, tag="msg")
                        if KSKIP_GATHER:
                            nc.gpsimd.memset(msg[:], 0.25)
                        else:
                            for k in range(NCHUNK):
                                rows_k = min(CHUNK, N - k * CHUNK)
                                L = int(Lseg[si, k])
                                o0 = int(seg_off[si, k]) - sb_off
                                for p0 in range(0, L, GMAX):
                                    pn = min(GMAX, L - p0)
                                    o = o0 + p0
                                    nc.gpsimd.dma_gather(
                                        out_ap=msg[:, o // 128:(o + pn) // 128, :],
                                        in_ap=src_full[k * CHUNK:k * CHUNK + rows_k, :],
                                        idxs_ap=gix[:, o // 16:(o + pn) // 16],
                                        num_idxs=pn, num_idxs_reg=pn, elem_size=w,
                                        single_packet=False)
                        if KSKIP_MM:
                            continue
                        # S partials per (chunk, lb) tile range (shifted iota)
                        Sp = {}
                        for k in range(NCHUNK):
                            for lb in range(len(sb)):
                                r = sranges[si].get((k, lb))
                                if r is None:
                                    continue
                                t0, t1 = r
                                ntl = t1 - t0 + 1
                                St = sbp.tile([128, ntl, 128], dt.bfloat16,
                                              tag=f"S{k}_{lb}", name=f"St{k}_{lb}")
                                nc.vector.scalar_tensor_tensor(
                                    out=St[:],
                                    in0=iota_t[:].unsqueeze(1).broadcast_to(
                                        [128, ntl, 128]),
                                    scalar=float(128 * lb),
                                    in1=dsl[:, t0:t1 + 1].unsqueeze(2).broadcast_to(
                                        [128, ntl, 128]),
                                    op0=Alu.add, op1=Alu.is_equal)
                                Sp[(k, lb)] = (St, t0)
                        if not is_bf:
                            m16 = sbp.tile([128, nt_sb, w], dt.bfloat16, tag="m16")
                            nc.vector.tensor_copy(out=m16[:], in_=msg[:])
                            mm = m16
                        else:
                            mm = msg
                        # dinvrep slab for the superblock (epilogue scaling)
                        c0 = sb[0] * 128
                        cn = min(NPC, (sb[-1] + 1) * 128) - c0
                        if i < 4:
                            drs = sbp.tile([128, SB_BLOCKS * 128], dt.float32,
                                           tag="drs")
                            nc.sync.dma_start(out=drs[:, :cn],
                                              in_=dre_d[:, c0:c0 + cn])
                        # own h2s rows for this sb (self-loop fold)
                        ost = sbp.tile([128, SB_BLOCKS, w], mdt, tag="ost")
                        if any(min(128, NPC - b * 128) < 128 for b in sb):
                            nc.vector.memset(ost[:], 0.0)
                        for lb, b in enumerate(sb):
                            rows = min(128, NPC - b * 128)
                            nc.sync.dma_start(
                                out=ost[:rows, lb, :],
                                in_=h2s[i][0][b * 128:b * 128 + rows, :])
                        if not is_bf:
                            o16 = sbp.tile([128, SB_BLOCKS, w], dt.bfloat16,
                                           tag="o16")
                            nc.vector.tensor_copy(out=o16[:], in_=ost[:])
                            ost_mm = o16
                        else:
                            ost_mm = ost
                        # aggregation matmuls (self-loop first, then tokens)
                        entries = sb_mm[si]
                        last_t = {}
                        for (t, lb, k) in entries:
                            last_t[lb] = (t, k)
                        pa = {}
                        for lb, b in enumerate(sb):
                            if i < 4:
                                pa[lb] = ps.tile([w, 128], dt.float32,
                                                 tag=f"agg{lb}", name=f"agg{lb}")
                                nc.tensor.matmul(pa[lb][:], ost_mm[:, lb, :],
                                                 id16_t[:], start=True,
                                                 stop=(lb not in last_t))
                            else:
                                pa[lb] = ps.tile([128, w], dt.float32,
                                                 tag=f"agg{lb}", name=f"agg5{lb}")
                                nc.tensor.matmul(pa[lb][:], id16_t[:],
                                                 ost_mm[:, lb, :], start=True,
                                                 stop=(lb not in last_t))
                        for (t, lb, k) in entries:
                            St, t0 = Sp[(k, lb)]
                            stop = (last_t[lb] == (t, k))
                            if i < 4:
                                nc.tensor.matmul(
                                    pa[lb][:], mm[:, t, :], St[:, t - t0, :],
                                    start=False, stop=stop)
                            else:
                                nc.tensor.matmul(
                                    pa[lb][:], St[:, t - t0, :], mm[:, t, :],
                                    start=False, stop=stop)
                        # epilogue per block
                        for lb, b in enumerate(sb):
                            rows = min(128, NPC - b * 128)
                            if i < 4:
                                # epilogue: hT = relu(pa * dinvrep + bias)
                                tmp = sbp.tile([w, 128], dt.float32, tag="tmp")
                                if rows < 128:
                                    nc.vector.memset(tmp[:, rows:], 0.0)
                                nc.vector.tensor_mul(
                                    tmp[:, :rows], pa[lb][:, :rows],
                                    drs[:w, lb * 128:lb * 128 + rows])
                                hT = sbp.tile([w, 128], dt.float32, tag="hT")
                                nc.scalar.activation(hT[:], tmp[:], Act.Relu,
                                                     bias=bias_t[i][:w, 0:1],
                                                     scale=1.0)
                                # transform: psum2 [128n, w2] = hT.T @ W_{i+1}
                                w2 = FW[i + 1]
                                pt = ps2.tile([128, w2], dt.float32, tag="tf")
                                nc.tensor.matmul(pt[:], hT[:], W_t[i + 1][:],
                                                 start=True, stop=True)
                                odt = (dt.bfloat16 if FDT[i + 1] == "bf16"
                                       else dt.float32)
                                st = sbp.tile([128, w2], odt, tag="h2st")
                                nc.vector.tensor_scalar(
                                    out=st[:], in0=pt[:],
                                    scalar1=dcol_t[:, b:b + 1],
                                    scalar2=None, op0=Alu.mult)
                                nc.sync.dma_start(
                                    out=h2s[i + 1][0][b * 128:b * 128 + rows, :],
                                    in_=st[:rows, :])
                            else:
                                # epilogue: h5 = relu(pa * dinvcol + b5rep)
                                tmp = sbp.tile([128, w], dt.float32, tag="tmp5")
                                nc.vector.scalar_tensor_tensor(
                                    out=tmp[:], in0=pa[lb][:],
                                    scalar=dcol_t[:, b:b + 1],
                                    in1=b5r_t[:], op0=Alu.mult, op1=Alu.add)
                                h5 = sbp.tile([128, w], dt.float32, tag="h5")
                                nc.scalar.activation(h5[:], tmp[:], Act.Relu)
                                # pooling: psum_pool [32, NG] += h5[:, :32].T @ B
                                h516 = sbp.tile([128, w], dt.bfloat16, tag="h516")
                                nc.vector.tensor_copy(out=h516[:], in_=h5[:])
                                B = sbp.tile([128, NG], dt.bfloat16, tag="B")
                                nc.vector.tensor_scalar(
                                    out=B[:], in0=iog_t[:],
                                    scalar1=bloc_t[:, b:b + 1],
                                    scalar2=None, op0=Alu.is_equal)
                                nc.tensor.matmul(
                                    psum_pool_t[:], h516[:, 0:32], B[:],
                                    start=(si == 0 and lb == 0),
                                    stop=(si == nsb - 1 and lb == len(sb) - 1))
                    if i < 4:
                        nc.gpsimd.collective_compute(
                            "AllGather", Alu.bypass, replica_groups=RG,
                            ins=[h2s[i + 1][0].opt()], outs=[h2s[i + 1][1].opt()])

                # ---- pooling tail: AllReduce, scale, FC ----
                pl = sbp.tile([32, NG], dt.float32, tag="pl")
                nc.vector.tensor_copy(out=pl[:], in_=psum_pool_t[:])
                nc.sync.dma_start(out=ar_i[:], in_=pl[:])
                nc.gpsimd.collective_compute(
                    "AllReduce", Alu.add, replica_groups=RG,
                    ins=[ar_i.opt()], outs=[ar_o.opt()])
                pls = sbp.tile([32, NG], dt.float32, tag="pls")
                nc.sync.dma_start(out=pls[:], in_=ar_o[:])
                plsc = sbp.tile([32, NG], dt.float32, tag="plsc")
                nc.vector.tensor_mul(plsc[:], pls[:], invc_t[:])
                pf = psp.tile([NG, 10], dt.float32, tag="fc")
                nc.tensor.matmul(pf[:], plsc[:], wfc_t[:], start=True, stop=True)
                ot = sbp.tile([NG, 10], dt.float32, tag="ot")
                nc.vector.tensor_add(ot[:], pf[:], bfc_t[:])
                nc.sync.dma_start(out=out_d[:], in_=ot[:])

    nc.compile()
    return nc


def kernel(x, edge_index, batch, W1, b1, W2, b2, W3, b3, W4, b4, W5, b5,
           Wfc, bfc):
    global _last_results, _last_nc, _last_in_maps
    from concourse.bass_utils import run_bass_kernel_spmd

    x = np.asarray(x, np.float32)
    edge_index = np.asarray(edge_index, np.int64)
    batch = np.asarray(batch, np.int64)

    dst_all = np.concatenate([edge_index[1], np.arange(N, dtype=np.int64)])
    deg = np.bincount(dst_all, minlength=N).astype(np.float32)
    dinv = np.where(deg > 0, 1.0 / np.sqrt(deg), 0.0).astype(np.float32)

    meta, core_data, node_data = _prep(x, edge_index, batch, dinv)

    # weights: W5/b5 padded to 64 outputs
    W5p = np.zeros((64, 64), np.float32)
    W5p[:, :32] = np.asarray(W5, np.float32)
    b5p = np.zeros(64, np.float32)
    b5p[:32] = np.asarray(b5, np.float32)
    Ws = [np.asarray(W1, np.float32), np.asarray(W2, np.float32),
          np.asarray(W3, np.float32), np.asarray(W4, np.float32), W5p]
    bs = []
    for b_ in (b1, b2, b3, b4):
        bp = np.zeros((128, 1), np.float32)
        v = np.asarray(b_, np.float32).ravel()
        bp[:v.shape[0], 0] = v
        bs.append(bp)
    b5rep = np.broadcast_to(b5p, (128, 64)).copy()
    cnt = np.bincount(batch, minlength=NG).astype(np.float32)
    invc = (1.0 / np.maximum(cnt, 1.0)).astype(np.float32)
    invc_rep = np.broadcast_to(invc, (32, NG)).copy()
    bfc_rep = np.broadcast_to(np.asarray(bfc, np.float32), (NG, 10)).copy()
    iota = np.broadcast_to(np.arange(128, dtype=np.float32), (128, 128)).copy()
    ident16 = np.eye(128, dtype=bf16)
    iota64 = np.broadcast_to(np.arange(NG, dtype=np.float32), (128, NG)).copy()

    nc = _build_program(meta, repeats=int(os.environ.get('KREPEATS', '1')))

    in_maps = []
    for c in range(NCORES):
        gidx_w, dstloc_w = core_data[c]
        xT, dre, dcol, bloc = node_data[c]
        im = {"xT": xT, "gidx": gidx_w, "dloc": dstloc_w, "dinvrep": dre,
              "dinvcol": dcol, "batchloc": bloc, "iota": iota,
              "ident16": ident16, "iota64": iota64,
              "b5rep": b5rep, "Wfc": np.asarray(Wfc, np.float32),
              "invcrep": invc_rep, "bfcrep": bfc_rep}
        for i in range(5):
            im[f"W{i+1}"] = Ws[i]
        for i in range(4):
            im[f"b{i+1}"] = bs[i]
        in_maps.append(im)

    _last_nc = nc
    _last_in_maps = in_maps
    res = run_bass_kernel_spmd(nc, in_maps, core_ids=list(range(NCORES)))
    _last_results = res
    return np.asarray(res.results[0]["out"], np.float32)


# revision 8
# speedup vs baseline: 1.2881x; 1.0553x over previous
"""GCN 5-layer message-passing kernel for 8 Trainium2 NeuronCores.

Strategy (node-sharded, dst-partitioned), v1:
- Core c owns dst nodes [c*12500, (c+1)*12500). Non-self-loop edges are
  routed to the core owning their dst. Self-loop contributions are folded
  into a per-block identity matmul on locally-resident own-slice data
  (no gather, no tokens for the 12500 self loops per core).
- GCN norm factors dinv[src]*dinv[dst] are folded: dinv[src] pre-scales the
  transformed feature table h2s = dinv * (h @ W) before the gather; dinv[dst]
  scales the per-dst-block epilogue.
- Per layer: each core computes its slice of h2s (transform matmul), cores
  AllGather the full table, then each core gathers rows for its edges via
  GPSIMD dma_gather (random HBM reads) and reduces them per dst block with
  one-hot segment matmuls accumulating in PSUM (scatter-free).
- Edges are sorted by (superblock, src-chunk, dst-block, src) and padded to
  a 128 multiple only per (superblock, chunk) segment (cross-core max), so
  padding overhead is ~8% instead of ~30%. Tiles may straddle dst blocks;
  the one-hot S for block lb is built only over lb's tile range with a
  shifted iota (iota + 128*lb == slot512), which self-masks other blocks.
- Final global mean-pool via one-hot matmul + AllReduce, then the FC layer.
"""
import os
import numpy as np
import ml_dtypes

KSKIP_GATHER = os.environ.get("KSKIP_GATHER", "0") == "1"
KSKIP_MM = os.environ.get("KSKIP_MM", "0") == "1"
GMAX = int(os.environ.get("KGMAX", "1024"))  # SWDGE ring cap per dma_gather

N = 100000
E = 1600000
NCORES = 8
NPC = N // NCORES            # 12500 nodes per core
NB = (NPC + 127) // 128      # 98 dst blocks per core
SB_BLOCKS = 4                # dst blocks per superblock
CHUNK = 32768                # gather src chunk (int16 index range)
NCHUNK = (N + CHUNK - 1) // CHUNK  # 4
NG = 64                      # graphs
F0 = 128
bf16 = ml_dtypes.bfloat16

# layer i: h2s_i width (transformed source), produced by W_i
FW = [128, 128, 128, 64, 64]   # W5 padded 32->64
# h2s dtype per layer: bf16 for 128-wide (256B elems), f32 for 64-wide (256B)
FDT = ["bf16", "bf16", "bf16", "f32", "f32"]

_last_results = None
_last_nc = None
_last_in_maps = None


def _superblocks():
    sbs = []
    b = 0
    while b < NB:
        sbs.append(list(range(b, min(b + SB_BLOCKS, NB))))
        b += SB_BLOCKS
    return sbs


def _prep(x, edge_index, batch, dinv):
    """Build common program structure + per-core data arrays.

    Token layout: (sb, chunk) segments, each padded to a 128 multiple of the
    cross-core max token count. Within a segment tokens are sorted by
    (dst block, src). Self-loop edges are NOT tokenized.
    """
    src = edge_index[0]
    dst = edge_index[1]

    sbs = _superblocks()
    nsb = len(sbs)

    # --- per-core sorted edges + per-(sb, chunk) counts ---
    core_edges = []
    counts = np.zeros((NCORES, nsb, NCHUNK), np.int64)
    for c in range(NCORES):
        lo = c * NPC
        sel = (dst >= lo) & (dst < lo + NPC)
        es = src[sel]
        ed = dst[sel] - lo
        blk = ed >> 7
        sb = blk // SB_BLOCKS
        slot = ed - sb * (SB_BLOCKS * 128)   # slot within superblock [0,512)
        chk = es >> 15
        order = np.lexsort((es, blk, chk, sb))
        es, sb, blk, slot, chk = (es[order], sb[order], blk[order],
                                  slot[order], chk[order])
        np.add.at(counts[c], (sb, chk), 1)
        core_edges.append((es, sb, blk, slot, chk))

    # segment lengths: cross-core max, padded to 128
    Lseg = ((counts.max(axis=0) + 127) // 128) * 128   # [nsb, NCHUNK]

    # --- traversal order: (sb, chunk) -> token offsets ---
    seg_off = np.zeros((nsb, NCHUNK), np.int64)
    tok = 0
    sb_tok_off = np.zeros(nsb, np.int64)
    for si in range(nsb):
        sb_tok_off[si] = tok
        for k in range(NCHUNK):
            seg_off[si, k] = tok
            tok += int(Lseg[si, k])
    T = tok

    # --- per-core data arrays in the common layout ---
    core_data = []
    boundaries = np.zeros((NCORES, nsb, NCHUNK, SB_BLOCKS + 1), np.int64)
    for c in range(NCORES):
        es, sb, blk, slot, chk = core_edges[c]
        key = sb * NCHUNK + chk
        if len(es):
            starts = np.r_[0, np.flatnonzero(np.diff(key)) + 1]
            runlen = np.diff(np.r_[starts, len(es)])
            rank = np.arange(len(es)) - np.repeat(starts, runlen)
        else:
            rank = np.zeros(0, np.int64)
        pos = seg_off[sb, chk] + rank
        idx16 = np.full(T, -1, np.int16)
        slotv = np.full(T, 999.0, np.float32)
        idx16[pos] = (es - chk * CHUNK).astype(np.int16)
        slotv[pos] = slot
        # per-call valid counts (valid tokens are a prefix of each call range)
        regs = []
        for si2 in range(nsb):
            for k2 in range(NCHUNK):
                L2 = int(Lseg[si2, k2])
                n2 = int(counts[c, si2, k2])
                for p0 in range(0, L2, GMAX):
                    pn = min(GMAX, L2 - p0)
                    v = min(max(n2 - p0, 0), pn)
                    if v == 0:
                        idx16[int(seg_off[si2, k2]) + p0] = 0
                        v = 1
                    regs.append(v)
        gcnt = np.asarray(regs, np.int32).reshape(1, -1)
        lb = blk - sb * SB_BLOCKS
        np.add.at(boundaries[c], (sb, chk, lb + 1), 1)
        gidx_w = np.tile(idx16.reshape(T // 16, 16).T, (8, 1))
        dstloc_w = slotv.reshape(T // 128, 128).T.copy()
        core_data.append((gidx_w, dstloc_w, gcnt))
    boundaries = np.cumsum(boundaries, axis=3)  # [c, sb, chk, lb+1] prefix

    # --- per (sb, chunk, lb): union tile range + matmul entries ---
    # sranges[si]: dict (k, lb) -> (t0, t1) sb-local tile range (inclusive)
    # sb_mm[si]: ordered list of (t_local, lb, k)
    sranges = []
    sb_mm = []
    for si in range(nsb):
        nblk = len(sbs[si])
        rng = {}
        entries = []
        for k in range(NCHUNK):
            base = int(seg_off[si, k] - sb_tok_off[si])
            for lb in range(nblk):
                s = int(boundaries[:, si, k, lb].min(axis=0))
                e = int(boundaries[:, si, k, lb + 1].max(axis=0))
                if e <= s:
                    continue
                t0 = (base + s) // 128
                t1 = (base + e - 1) // 128
                rng[(k, lb)] = (t0, t1)
                for t in range(t0, t1 + 1):
                    entries.append((t, lb, k))
        entries.sort()
        sranges.append(rng)
        sb_mm.append(entries)

    # --- per-core node-level arrays ---
    node_data = []
    for c in range(NCORES):
        lo = c * NPC
        xT = np.ascontiguousarray(x[lo:lo + NPC].T)              # [128, NPC]
        dre = np.broadcast_to(dinv[lo:lo + NPC], (128, NPC)).copy()
        dpad = np.ones(NB * 128, np.float32)
        dpad[:NPC] = dinv[lo:lo + NPC]
        dcol = dpad.reshape(NB, 128).T.copy()                    # [128, NB]
        bpad = np.full(NB * 128, 999.0, np.float32)
        bpad[:NPC] = batch[lo:lo + NPC].astype(np.float32)
        bloc = bpad.reshape(NB, 128).T.copy()                    # [128, NB]
        node_data.append((xT, dre, dcol, bloc))

    ncalls = sum((int(L) + GMAX - 1) // GMAX for L in Lseg.ravel() if L)
    meta = dict(sbs=sbs, Lseg=Lseg, seg_off=seg_off, sb_tok_off=sb_tok_off,
                sranges=sranges, sb_mm=sb_mm, T=T, ncalls=ncalls)
    return meta, core_data, node_data


def _build_program(meta, repeats=1):
    import concourse.bass as bass
    import concourse.bacc as bacc
    import concourse.tile as tile
    from concourse import mybir
    dt = mybir.dt
    Alu = mybir.AluOpType
    Act = mybir.ActivationFunctionType

    T = meta["T"]
    sbs = meta["sbs"]
    seg_off = meta["seg_off"]
    sb_tok_off = meta["sb_tok_off"]
    Lseg = meta["Lseg"]
    sranges = meta["sranges"]
    sb_mm = meta["sb_mm"]
    nsb = len(sbs)

    nc = bacc.Bacc("TRN2", target_bir_lowering=False, debug=False,
                   num_devices=NCORES)

    # ---- IO ----
    xT_d = nc.dram_tensor("xT", [128, NPC], dt.float32, kind="ExternalInput")
    gidx_d = nc.dram_tensor("gidx", [128, T // 16], dt.int16, kind="ExternalInput")
    gcnt_d = nc.dram_tensor("gcnt", [1, meta["ncalls"]], dt.int32,
                            kind="ExternalInput")
    dloc_d = nc.dram_tensor("dloc", [128, T // 128], dt.float32, kind="ExternalInput")
    dre_d = nc.dram_tensor("dinvrep", [128, NPC], dt.float32, kind="ExternalInput")
    dcol_d = nc.dram_tensor("dinvcol", [128, NB], dt.float32, kind="ExternalInput")
    bloc_d = nc.dram_tensor("batchloc", [128, NB], dt.float32, kind="ExternalInput")
    iota_d = nc.dram_tensor("iota", [128, 128], dt.float32, kind="ExternalInput")
    id16_d = nc.dram_tensor("ident16", [128, 128], dt.bfloat16, kind="ExternalInput")
    iog_d = nc.dram_tensor("iota64", [128, NG], dt.float32, kind="ExternalInput")
    W_d = [nc.dram_tensor(f"W{i+1}", [128 if i == 0 else FW[i - 1], FW[i]],
                          dt.float32, kind="ExternalInput") for i in range(5)]
    bias_d = [nc.dram_tensor(f"b{i+1}", [128, 1], dt.float32, kind="ExternalInput")
              for i in range(4)]
    b5r_d = nc.dram_tensor("b5rep", [128, 64], dt.float32, kind="ExternalInput")
    wfc_d = nc.dram_tensor("Wfc", [32, 10], dt.float32, kind="ExternalInput")
    invc_d = nc.dram_tensor("invcrep", [32, NG], dt.float32, kind="ExternalInput")
    bfc_d = nc.dram_tensor("bfcrep", [NG, 10], dt.float32, kind="ExternalInput")
    out_d = nc.dram_tensor("out", [NG, 10], dt.float32, kind="ExternalOutput")

    RG = [list(range(NCORES))]

    with tile.TileContext(nc) as tc:
        with tc.tile_pool(name="cst", bufs=1) as cst, \
             tc.tile_pool(name="sb", bufs=2) as sbp, \
             tc.tile_pool(name="ps", bufs=1, space="PSUM") as ps, \
             tc.tile_pool(name="ps2", bufs=2, space="PSUM") as ps2, \
             tc.tile_pool(name="psp", bufs=1, space="PSUM") as psp, \
             tc.tile_pool(name="dram", bufs=1, space="DRAM") as dram:

            # ---- constants to SBUF ----
            def cload(name, dten, shape, dtype):
                t = cst.tile(shape, dtype, tag=name)
                nc.sync.dma_start(out=t[:], in_=dten[:])
                return t
            iota_t = cload("iota", iota_d, [128, 128], dt.float32)
            id16_t = cload("id16", id16_d, [128, 128], dt.bfloat16)
            iog_t = cload("iog", iog_d, [128, NG], dt.float32)
            W_t = [cload(f"W{i}", W_d[i], list(W_d[i].shape), dt.float32)
                   for i in range(5)]
            bias_t = [cload(f"b{i}", bias_d[i], [128, 1], dt.float32)
                      for i in range(4)]
            b5r_t = cload("b5r", b5r_d, [128, 64], dt.float32)
            wfc_t = cload("wfc", wfc_d, [32, 10], dt.float32)
            invc_t = cload("invc", invc_d, [32, NG], dt.float32)
            bfc_t = cload("bfc", bfc_d, [NG, 10], dt.float32)
            dcol_t = cload("dcol", dcol_d, [128, NB], dt.float32)
            bloc_t = cload("bloc", bloc_d, [128, NB], dt.float32)
            gcnt_t = cload("gcnt", gcnt_d, [1, meta["ncalls"]], dt.int32)

            # ---- DRAM internals ----
            def h2s_tiles(i):
                w, d = FW[i], (dt.bfloat16 if FDT[i] == "bf16" else dt.float32)
                own = dram.tile([NPC, w], d, tag=f"own{i}")
                full = dram.tile([N, w], d, tag=f"full{i}")
                return own, full
            h2s = [h2s_tiles(i) for i in range(5)]
            ar_i = dram.tile([32, NG], dt.float32, tag="ari")
            ar_o = dram.tile([32, NG], dt.float32, tag="aro")

            psum_pool_t = psp.tile([32, NG], dt.float32, tag="pool")
            for _rep in range(repeats):

                # ---- layer 1 transform: h2s_0 = dinv * (x @ W1), bf16 ----
                for si, sb in enumerate(sbs):
                    c0 = sb[0] * 128
                    cn = min(NPC, (sb[-1] + 1) * 128) - c0
                    xsl = sbp.tile([128, SB_BLOCKS * 128], dt.float32, tag="xsl")
                    nc.sync.dma_start(out=xsl[:, :cn], in_=xT_d[:, c0:c0 + cn])
                    for lb, b in enumerate(sb):
                        rows = min(128, NPC - b * 128)
                        pt = ps2.tile([128, FW[0]], dt.float32, tag="tf")
                        nc.tensor.matmul(pt[:], xsl[:, lb * 128:lb * 128 + 128],
                                         W_t[0][:], start=True, stop=True)
                        st = sbp.tile([128, FW[0]], dt.bfloat16, tag="h2st")
                        nc.vector.tensor_scalar(
                            out=st[:], in0=pt[:], scalar1=dcol_t[:, b:b + 1],
                            scalar2=None, op0=Alu.mult)
                        nc.sync.dma_start(out=h2s[0][0][b * 128:b * 128 + rows, :],
                                          in_=st[:rows, :])
                nc.gpsimd.collective_compute(
                    "AllGather", Alu.bypass, replica_groups=RG,
                    ins=[h2s[0][0].opt()], outs=[h2s[0][1].opt()])

                # ---- layers 1..5: gather + segment-matmul + epilogue ----
                greg = nc.gpsimd.alloc_register("gcnt_reg")
                for i in range(5):
                    w = FW[i]
                    is_bf = FDT[i] == "bf16"
                    mdt = dt.bfloat16 if is_bf else dt.float32
                    src_full = h2s[i][1]
                    call_idx = 0
                    for si, sb in enumerate(sbs):
                        sb_off = int(sb_tok_off[si])
                        ntok = int(Lseg[si].sum())
                        nt_sb = ntok // 128
                        # metadata slabs
                        gix = sbp.tile([128, ntok // 16], dt.int16, tag="gix")
                        nc.sync.dma_start(
                            out=gix[:], in_=gidx_d[:, sb_off // 16:(sb_off + ntok) // 16])
                        dsl = sbp.tile([128, nt_sb], dt.float32, tag="dsl")
                        nc.sync.dma_start(
                            out=dsl[:], in_=dloc_d[:, sb_off // 128:(sb_off + ntok) // 128])
                        # gathers per (chunk) segment, GMAX-batched
                        msg = sbp.tile([128, nt_sb, w], mdt, tag="msg")
                        if KSKIP_GATHER:
                            nc.gpsimd.memset(msg[:], 0.25)
                        else:
                            for k in range(NCHUNK):
                                rows_k = min(CHUNK, N - k * CHUNK)
                                L = int(Lseg[si, k])
                                o0 = int(seg_off[si, k]) - sb_off
                                for p0 in range(0, L, GMAX):
                                    pn = min(GMAX, L - p0)
                                    o = o0 + p0
                                    nc.gpsimd.reg_load(
                                        greg, gcnt_t[0:1, call_idx:call_idx + 1])
                                    call_idx += 1
                                    nc.gpsimd.dma_gather(
                                        out_ap=msg[:, o // 128:(o + pn) // 128, :],
                                        in_ap=src_full[k * CHUNK:k * CHUNK + rows_k, :],
                                        idxs_ap=gix[:, o // 16:(o + pn) // 16],
                                        num_idxs=pn, num_idxs_reg=greg, elem_size=w,
                                        single_packet=False)
                        if KSKIP_MM:
                            continue
                        # S partials per (chunk, lb) tile range (shifted iota)
                        Sp = {}
                        for k in range(NCHUNK):
                            for lb in range(len(sb)):
                                r = sranges[si].get((k, lb))
                                if r is None:
                                    continue
                                t0, t1 = r
                                ntl = t1 - t0 + 1
                                St = sbp.tile([128, ntl, 128], dt.bfloat16,
                                              tag=f"S{k}_{lb}", name=f"St{k}_{lb}")
                                nc.vector.scalar_tensor_tensor(
                                    out=St[:],
                                    in0=iota_t[:].unsqueeze(1).broadcast_to(
                                        [128, ntl, 128]),
                                    scalar=float(128 * lb),
                                    in1=dsl[:, t0:t1 + 1].unsqueeze(2).broadcast_to(
                                        [128, ntl, 128]),
                                    op0=Alu.add, op1=Alu.is_equal)
                                Sp[(k, lb)] = (St, t0)
                        if not is_bf:
                            m16 = sbp.tile([128, nt_sb, w], dt.bfloat16, tag="m16")
                            nc.vector.tensor_copy(out=m16[:], in_=msg[:])
                            mm = m16
                        else:
                            mm = msg
                        # dinvrep slab for the superblock (epilogue scaling)
                        c0 = sb[0] * 128
                        cn = min(NPC, (sb[-1] + 1) * 128) - c0
                        if i < 4:
                            drs = sbp.tile([128, SB_BLOCKS * 128], dt.float32,
                                           tag="drs")
                            nc.sync.dma_start(out=drs[:, :cn],
                                              in_=dre_d[:, c0:c0 + cn])
                        # own h2s rows for this sb (self-loop fold)
                        ost = sbp.tile([128, SB_BLOCKS, w], mdt, tag="ost")
                        if any(min(128, NPC - b * 128) < 128 for b in sb):
                            nc.vector.memset(ost[:], 0.0)
                        for lb, b in enumerate(sb):
                            rows = min(128, NPC - b * 128)
                            nc.sync.dma_start(
                                out=ost[:rows, lb, :],
                                in_=h2s[i][0][b * 128:b * 128 + rows, :])
                        if not is_bf:
                            o16 = sbp.tile([128, SB_BLOCKS, w], dt.bfloat16,
                                           tag="o16")
                            nc.vector.tensor_copy(out=o16[:], in_=ost[:])
                            ost_mm = o16
                        else:
                            ost_mm = ost
                        # aggregation matmuls (self-loop first, then tokens)
                        entries = sb_mm[si]
                        last_t = {}
                        for (t, lb, k) in entries:
                            last_t[lb] = (t, k)
                        pa = {}
                        for lb, b in enumerate(sb):
                            if i < 4:
                                pa[lb] = ps.tile([w, 128], dt.float32,
                                                 tag=f"agg{lb}", name=f"agg{lb}")
                                nc.tensor.matmul(pa[lb][:], ost_mm[:, lb, :],
                                                 id16_t[:], start=True,
                                                 stop=(lb not in last_t))
                            else:
                                pa[lb] = ps.tile([128, w], dt.float32,
                                                 tag=f"agg{lb}", name=f"agg5{lb}")
                                nc.tensor.matmul(pa[lb][:], id16_t[:],
                                                 ost_mm[:, lb, :], start=True,
                                                 stop=(lb not in last_t))
                        for (t, lb, k) in entries:
                            St, t0 = Sp[(k, lb)]
                            stop = (last_t[lb] == (t, k))
                            if i < 4:
                                nc.tensor.matmul(
                                    pa[lb][:], mm[:, t, :], St[:, t - t0, :],
                                    start=False, stop=stop)
                            else:
                                nc.tensor.matmul(
                                    pa[lb][:], St[:, t - t0, :], mm[:, t, :],
                                    start=False, stop=stop)
                        # epilogue per block
                        for lb, b in enumerate(sb):
                            rows = min(128, NPC - b * 128)
                            if i < 4:
                                # epilogue: hT = relu(pa * dinvrep + bias)
                                tmp = sbp.tile([w, 128], dt.float32, tag="tmp")
                                if rows < 128:
                                    nc.vector.memset(tmp[:, rows:], 0.0)
                                nc.vector.tensor_mul(
                                    tmp[:, :rows], pa[lb][:, :rows],
                                    drs[:w, lb * 128:lb * 128 + rows])
                                hT = sbp.tile([w, 128], dt.float32, tag="hT")
                                nc.scalar.activation(hT[:], tmp[:], Act.Relu,
                                                     bias=bias_t[i][:w, 0:1],
                                                     scale=1.0)
                                # transform: psum2 [128n, w2] = hT.T @ W_{i+1}
                                w2 = FW[i + 1]
                                pt = ps2.tile([128, w2], dt.float32, tag="tf")
                                nc.tensor.matmul(pt[:], hT[:], W_t[i + 1][:],
                                                 start=True, stop=True)
                                odt = (dt.bfloat16 if FDT[i + 1] == "bf16"
                                       else dt.float32)
                                st = sbp.tile([128, w2], odt, tag="h2st")
                                nc.vector.tensor_scalar(
                                    out=st[:], in0=pt[:],
                                    scalar1=dcol_t[:, b:b + 1],
                                    scalar2=None, op0=Alu.mult)
                                nc.sync.dma_start(
                                    out=h2s[i + 1][0][b * 128:b * 128 + rows, :],
                                    in_=st[:rows, :])
                            else:
                                # epilogue: h5 = relu(pa * dinvcol + b5rep)
                                tmp = sbp.tile([128, w], dt.float32, tag="tmp5")
                                nc.vector.scalar_tensor_tensor(
                                    out=tmp[:], in0=pa[lb][:],
                                    scalar=dcol_t[:, b:b + 1],
                                    in1=b5r_t[:], op0=Alu.mult, op1=Alu.add)
                                h5 = sbp.tile([128, w], dt.float32, tag="h5")
                                nc.scalar.activation(h5[:], tmp[:], Act.Relu)
                                # pooling: psum_pool [32, NG] += h5[:, :32].T @ B
                                h516 = sbp.tile([128, w], dt.bfloat16, tag="h516")
                                nc.vector.tensor_copy(out=h516[:], in_=h5[:])
                                B = sbp.tile([128, NG], dt.bfloat16, tag="B")
                                nc.vector.tensor_scalar(
                                    out=B[:], in0=iog_t[:],
                                    scalar1=bloc_t[:, b:b + 1],
                                    scalar2=None, op0=Alu.is_equal)
                                nc.tensor.matmul(
                                    psum_pool_t[:], h516[:, 0:32], B[:],
                                    start=(si == 0 and lb == 0),
                                    stop=(si == nsb - 1 and lb == len(sb) - 1))
                    if i < 4:
                        nc.gpsimd.collective_compute(
                            "AllGather", Alu.bypass, replica_groups=RG,
                            ins=[h2s[i + 1][0].opt()], outs=[h2s[i + 1][1].opt()])

                nc.gpsimd.free_register(greg)

                # ---- pooling tail: AllReduce, scale, FC ----
                pl = sbp.tile([32, NG], dt.float32, tag="pl")
                nc.vector.tensor_copy(out=pl[:], in_=psum_pool_t[:])
                nc.sync.dma_start(out=ar_i[:], in_=pl[:])
                nc.gpsimd.collective_compute(
                    "AllReduce", Alu.add, replica_groups=RG,
                    ins=[ar_i.opt()], outs=[ar_o.opt()])
                pls = sbp.tile([32, NG], dt.float32, tag="pls")
                nc.sync.dma_start(out=pls[:], in_=ar_o[:])
                plsc = sbp.tile([32, NG], dt.float32, tag="plsc")
                nc.vector.tensor_mul(plsc[:], pls[:], invc_t[:])
                pf = psp.tile([NG, 10], dt.float32, tag="fc")
                nc.tensor.matmul(pf[:], plsc[:], wfc_t[:], start=True, stop=True)
                ot = sbp.tile([NG, 10], dt.float32, tag="ot")
                nc.vector.tensor_add(ot[:], pf[:], bfc_t[:])
                nc.sync.dma_start(out=out_d[:], in_=ot[:])

    nc.compile()
    return nc


def kernel(x, edge_index, batch, W1, b1, W2, b2, W3, b3, W4, b4, W5, b5,
           Wfc, bfc):
    global _last_results, _last_nc, _last_in_maps
    from concourse.bass_utils import run_bass_kernel_spmd

    x = np.asarray(x, np.float32)
    edge_index = np.asarray(edge_index, np.int64)
    batch = np.asarray(batch, np.int64)

    dst_all = np.concatenate([edge_index[1], np.arange(N, dtype=np.int64)])
    deg = np.bincount(dst_all, minlength=N).astype(np.float32)
    dinv = np.where(deg > 0, 1.0 / np.sqrt(deg), 0.0).astype(np.float32)

    meta, core_data, node_data = _prep(x, edge_index, batch, dinv)

    # weights: W5/b5 padded to 64 outputs
    W5p = np.zeros((64, 64), np.float32)
    W5p[:, :32] = np.asarray(W5, np.float32)
    b5p = np.zeros(64, np.float32)
    b5p[:32] = np.asarray(b5, np.float32)
    Ws = [np.asarray(W1, np.float32), np.asarray(W2, np.float32),
          np.asarray(W3, np.float32), np.asarray(W4, np.float32), W5p]
    bs = []
    for b_ in (b1, b2, b3, b4):
        bp = np.zeros((128, 1), np.float32)
        v = np.asarray(b_, np.float32).ravel()
        bp[:v.shape[0], 0] = v
        bs.append(bp)
    b5rep = np.broadcast_to(b5p, (128, 64)).copy()
    cnt = np.bincount(batch, minlength=NG).astype(np.float32)
    invc = (1.0 / np.maximum(cnt, 1.0)).astype(np.float32)
    invc_rep = np.broadcast_to(invc, (32, NG)).copy()
    bfc_rep = np.broadcast_to(np.asarray(bfc, np.float32), (NG, 10)).copy()
    iota = np.broadcast_to(np.arange(128, dtype=np.float32), (128, 128)).copy()
    ident16 = np.eye(128, dtype=bf16)
    iota64 = np.broadcast_to(np.arange(NG, dtype=np.float32), (128, NG)).copy()

    nc = _build_program(meta, repeats=int(os.environ.get('KREPEATS', '1')))

    in_maps = []
    for c in range(NCORES):
        gidx_w, dstloc_w, gcnt = core_data[c]
        xT, dre, dcol, bloc = node_data[c]
        im = {"xT": xT, "gidx": gidx_w, "dloc": dstloc_w, "gcnt": gcnt,
              "dinvrep": dre,
              "dinvcol": dcol, "batchloc": bloc, "iota": iota,
              "ident16": ident16, "iota64": iota64,
              "b5rep": b5rep, "Wfc": np.asarray(Wfc, np.float32),
              "invcrep": invc_rep, "bfcrep": bfc_rep}
        for i in range(5):
            im[f"W{i+1}"] = Ws[i]
        for i in range(4):
            im[f"b{i+1}"] = bs[i]
        in_maps.append(im)

    _last_nc = nc
    _last_in_maps = in_maps
    res = run_bass_kernel_spmd(nc, in_maps, core_ids=list(range(NCORES)))
    _last_results = res
    return np.asarray(res.results[0]["out"], np.float32)
